# revision 26
# baseline (speedup 1.0000x reference)
"""Bayesian-router MoE kernel for 8 Trainium2 NeuronCores.

Strategy (expert-parallel, per sharding hint):
  - Router moments / top-k / combine weights: tiny (B*F*E ~ 17 MFLOP), computed
    on host in float64 (min score gap ~1.7e-4, far above fp32 noise, so expert
    selection is stable vs the fp32 reference).
  - Token dispatch: host gathers each expert's routed tokens into a padded,
    transposed buffer XgT [F, CAP] (the host-side equivalent of the
    all-to-all; full I/O contract means shard/unshard happens on host).
    Experts are sorted by token count: the 8 largest go to slot 0 (cap0),
    the 8 smallest to slot 1 (cap1 <= cap0), one of each per core.
  - Device: each of the 8 cores runs its 2-expert MLP on gathered tokens in
    transposed form (A1T = relu(W1^T XgT + b1), YT = W2^T A1T + b2) so no
    on-device transposes are needed. Inputs/activations ship+multiply as fp16
    (PSUM accumulation stays fp32): 1 col/cycle on the PE, 4x fp32 rate.
  - Schedule (rewrite of the ~50us baseline, which had a dense MM stream
    but a ~5us DMA ramp, a ~2.7us HAM cold-clock penalty, and a ~6.4us
    serialized output tail):
      * 8 warm-up matmuls on a memset tile cycle the 8-deep PSUM ring
        (no WAW chaining) and keep the PE busy from the moment the engines
        leave the framework preamble, so the HAM clock-gate is at 8/8
        (2.4 GHz) when the real stream starts instead of ~10us later.
      * weights are host-swizzled to [P, mblock, k, 128] so every weight
        DMA moves >=1KB contiguous runs per partition at line rate, and
        per-m-block transfers complete in the order layer 1 consumes them
        (SDMA round-robins all queued transfers of a ring, so completion
        order tracks issue granularity, and every transfer pays a ~2us
        completion-receipt before its semaphore fires).
      * bulk transfers needed later (second token chunk, W2, all slot-1
        data) are gated behind early evictions so they cannot round-robin-
        steal HBM bandwidth from the ramp-critical pieces.  Completion-
        chaining instead would serialize the ~2us receipts; free-running
        them starves the ramp (both measured).
      * tokens are processed in <=320-column chunks: the L1->L2 boundary
        of a slot then only waits for one chunk's evictions, and eviction/
        output-DMA pipelining stays smooth through the end of the kernel.
      * layer-2 output is evicted as fp16 (halves out-traffic; ~2e-4 extra
        rounding) and each (chunk, m) tile is DMA'd on the otherwise-idle
        sync ring the moment it's evicted -- the baseline queued these
        behind activations on the scalar ring, serializing the tail.
    Remaining fixed costs (measured): ~3.4us engine preamble before the
    first DMA can issue, ~2us DMA completion receipt on the last output
    tile, and ~7us framework postamble that zeroes each engine's ~50-
    semaphore file.  The ~29.3us matmul stream itself is fp16 PE-clock
    bound; the chip's P0 power state (2.0 vs 2.4 GHz, run-to-run) moves
    the total by ~15%.
  - Combine: host scatter-adds w[t,e] * Y_e rows into the output (the
    cross-device reduction of the unshard step).
"""

import os
import numpy as np

NCORES = 8
P = 128
TOP_K = 4
N_WARMUP = 6


# ---------------------------------------------------------------------------
# host-side routing (matches reference math; float64 for stable ordering)
# ---------------------------------------------------------------------------
def _routing(h, W_mu, b_mu, W_logvar, b_logvar):
    h64 = h.astype(np.float64)
    mu = h64 @ W_mu.T.astype(np.float64) + b_mu.astype(np.float64)
    var = (h64 * h64) @ np.exp(W_logvar.astype(np.float64)).T + np.exp(
        b_logvar.astype(np.float64)
    )
    var = np.maximum(var, 1e-12)
    tilde = mu / np.sqrt(1.0 + (np.pi / 8.0) * var)
    t = tilde - tilde.max(axis=1, keepdims=True)
    ex = np.exp(t)
    probs = ex / ex.sum(axis=1, keepdims=True)
    idx = np.argsort(-tilde, axis=1, kind="stable")[:, :TOP_K]
    w = np.take_along_axis(probs, idx, axis=1)
    w = w / np.maximum(w.sum(axis=1, keepdims=True), 1e-12)
    return idx, w


def _chunks(cap, piece=320):
    # chunks of <=piece tokens: fine enough that eviction/DMA pipelining is
    # smooth and the L1->L2 boundary of a slot only waits for one chunk's
    # evictions, coarse enough that matmuls stay streaming-efficient
    n = max(1, -(-cap // piece))
    base, rem = divmod(cap, n)
    out = []
    off = 0
    for i in range(n):
        sz = base + (1 if i < rem else 0)
        out.append((off, sz))
        off += sz
    return out


# ---------------------------------------------------------------------------
# device kernel: 2-expert MLP on pre-gathered transposed tokens
# ---------------------------------------------------------------------------
def _build_kernel(F, H, C, caps):
    import concourse.mybir as mybir
    import concourse.tile as tile
    from concourse import bacc

    f32 = mybir.dt.float32
    f16 = mybir.dt.float16
    FK, HK, CK = F // P, H // P, C // P
    nslots = len(caps)
    # the DMA emission below hardcodes this problem's geometry
    assert (FK, HK, CK, nslots) == (4, 8, 4, 2), (FK, HK, CK, nslots)

    nc = bacc.Bacc("TRN2", target_bir_lowering=False, debug=False,
                   num_devices=NCORES)

    xts_d = [nc.dram_tensor(f"xt{s}", [F, caps[s]], f16, kind="ExternalInput")
             for s in range(nslots)]
    yts_d = [nc.dram_tensor(f"yt{s}", [C, caps[s]], f16, kind="ExternalOutput")
             for s in range(nslots)]
    # weights host-swizzled to [P, mblock, k, 128]: per-partition runs are
    # k*128 contiguous fp16 (>=1KB) and one m-block is an independent 128KB
    # transfer whose arrival unlocks compute for that block.
    w1_d = nc.dram_tensor("w1", [nslots, P, HK, FK, P], f16,
                          kind="ExternalInput")
    w2_d = nc.dram_tensor("w2", [nslots, P, CK, HK, P], f16,
                          kind="ExternalInput")
    b1_d = nc.dram_tensor("b1", [P, nslots, HK], f32, kind="ExternalInput")
    b2_d = nc.dram_tensor("b2", [P, nslots, CK], f32, kind="ExternalInput")

    add, amax = mybir.AluOpType.add, mybir.AluOpType.max

    with tile.TileContext(nc) as tc:
        with (
            tc.tile_pool(name="sb", bufs=1) as sb,
            tc.tile_pool(name="pschunk", bufs=1, space="PSUM") as pp,
        ):
            # --- PE warm-up: junk matmuls from ~7us keep the array busy
            # while the first real operands stream in, so the HAM clock-gate
            # opens to 8/8 right as the real stream begins.  memset FIRST on
            # gpsimd (before the b1/b2 DMA issues, which cost ~0.8us each
            # on that queue).  The warm-ups cycle the SAME 8-deep "ps" PSUM
            # ring the real groups use: no two consecutive warm-ups touch
            # the same bank, so they run back-to-back with no semaphore
            # chaining (a WAW chain would pace them at ~640ns+ each).
            wm = sb.tile([P, 512], f16)
            nc.gpsimd.memset(wm[:], 0.0)
            warms = []
            for _ in range(N_WARMUP):
                ps_w = pp.tile([P, 512], f32, tag="ps", bufs=8)
                warms.append(nc.tensor.matmul(ps_w[:], wm[:, :P], wm[:],
                                              start=True, stop=True))
            last_warm = warms[-1]

            # --- consts (gpsimd SWDGE ring; tiny) ---
            b1s = sb.tile([P, nslots, HK], f32)
            nc.gpsimd.dma_start(out=b1s[:], in_=b1_d[:])
            b2s = sb.tile([P, nslots, CK], f32)
            nc.gpsimd.dma_start(out=b2s[:], in_=b2_d[:])

            # --- SBUF tiles ---
            xts = [sb.tile([P, FK, caps[s]], f16, tag=f"xt{s}",
                           name=f"xts{s}") for s in range(nslots)]
            w1s = [sb.tile([P, HK, FK, P], f16, tag=f"w1_{s}",
                           name=f"w1s{s}") for s in range(nslots)]
            w2s = [sb.tile([P, CK, HK, P], f16, tag=f"w2_{s}",
                           name=f"w2s{s}") for s in range(nslots)]
            a1s = [sb.tile([P, HK, caps[s]], f16, tag=f"a1_{s}",
                           name=f"a1s{s}") for s in range(nslots)]
            ysb = [sb.tile([P, CK, caps[s]], f16, tag=f"yt_{s}",
                           name=f"ysb{s}") for s in range(nslots)]
            xt_r = [xts_d[s].rearrange("(k p) n -> p k n", p=P)
                    for s in range(nslots)]
            yt_r = [yts_d[s].rearrange("(k p) n -> p k n", p=P)
                    for s in range(nslots)]

            # --- input DMAs.  Ramp-critical transfers (first token chunk,
            # first W1 m-blocks) issue immediately in FIFO order, split
            # small so the first matmul group's operands land after ~200KB.
            # Bulk transfers needed later (W2, slot-1 weights/tokens) are
            # GATED behind early layer-1 evictions: completion-chaining
            # them instead would serialize on the ~2us DMA completion
            # receipt per link (measured), and letting them issue freely
            # would steal HBM bandwidth from the ramp.  scalar ring:
            # tokens; sync ring: weights (+ yt evacuations appended
            # later by the compute loop). ---
            gated = []

            def gate(dma, ev_idx):
                gated.append((dma, ev_idx))
                return dma

            def slot_chunks(s):
                cap = caps[s]
                if s == 0 and cap > 320 and cap - 320 >= 256:
                    return [(0, 320)] + [(320 + o, n)
                                         for o, n in _chunks(cap - 320)]
                return _chunks(cap)

            c0 = slot_chunks(0)[0][1]  # first chunk of slot 0
            nc.scalar.dma_start(out=xts[0][:, :2, :c0],
                                in_=xt_r[0][:, :2, :c0])
            nc.sync.dma_start(out=w1s[0][:, 0, 0:2], in_=w1_d[0][:, 0, 0:2])
            nc.scalar.dma_start(out=xts[0][:, 2:, :c0],
                                in_=xt_r[0][:, 2:, :c0])
            nc.sync.dma_start(out=w1s[0][:, 0, 2:4], in_=w1_d[0][:, 0, 2:4])
            # per-m-block w1 pieces: SDMA round-robins among all queued
            # transfers of a ring, so completion order ~ issue order only at
            # matching granularity -- block m lands just before group m
            # needs it
            nc.sync.dma_start(out=w1s[0][:, 1], in_=w1_d[0][:, 1])
            nc.sync.dma_start(out=w1s[0][:, 2], in_=w1_d[0][:, 2])
            nc.sync.dma_start(out=w1s[0][:, 3], in_=w1_d[0][:, 3])
            nc.sync.dma_start(out=w1s[0][:, 4:6], in_=w1_d[0][:, 4:6])
            nc.sync.dma_start(out=w1s[0][:, 6:8], in_=w1_d[0][:, 6:8])
            if c0 < caps[0]:
                # second token chunk of slot 0 is not needed until ~16us:
                # release it mid-warm-up (a Tensor-engine target cannot
                # deadlock the scalar ring this DMA parks on) so it does
                # not round-robin-steal HBM bandwidth from the
                # ramp-critical w1 blocks but still lands in time
                xt0c1_dma = nc.scalar.dma_start(out=xts[0][:, :, c0:],
                                                in_=xt_r[0][:, :, c0:])
            # gate indices chosen so each bulk release happens after the
            # slot-0 L1 ramp no longer needs exclusive bandwidth, but a few
            # us (transfer + ~2us completion receipt) before the consumer
            # phase starts
            # w2 of slot 0 releases at the very first evictions: the ramp
            # transfers are all done by then, so it gets a clean-bandwidth
            # window and lands (incl. ~2us receipt) well before layer 2.
            gate(nc.sync.dma_start(out=w2s[0][:, 0:2], in_=w2_d[0][:, 0:2]),
                 4)
            gate(nc.sync.dma_start(out=w2s[0][:, 2:4], in_=w2_d[0][:, 2:4]),
                 5)
            for s in range(1, nslots):
                # slot-1 bulk (3.5MB) waits until slot-0 L2 has started
                # (eviction #16 = first L2 eviction) so it never round-robin
                # -steals from w2s0, then streams in fine staggered pieces
                # that complete in consumption order.  All on the sync ring:
                # a gated DMA parked at the head of the scalar ring would
                # block the ACT evictions that share that queue.
                gate(nc.sync.dma_start(out=xts[s][:], in_=xt_r[s][:]), 16)
                gate(nc.sync.dma_start(out=w1s[s][:, 0:2],
                                       in_=w1_d[s][:, 0:2]), 17)
                gate(nc.sync.dma_start(out=w1s[s][:, 2:4],
                                       in_=w1_d[s][:, 2:4]), 18)
                gate(nc.sync.dma_start(out=w1s[s][:, 4:6],
                                       in_=w1_d[s][:, 4:6]), 19)
                gate(nc.sync.dma_start(out=w1s[s][:, 6:8],
                                       in_=w1_d[s][:, 6:8]), 20)
                gate(nc.sync.dma_start(out=w2s[s][:, 0:2],
                                       in_=w2_d[s][:, 0:2]), 22)
                gate(nc.sync.dma_start(out=w2s[s][:, 2:4],
                                       in_=w2_d[s][:, 2:4]), 24)

            # --- compute ---
            evs = []
            fillers = []

            def evict(dst, src, bias, relu):
                # alternate PSUM evictions between Scalar(ACT) and
                # Vector(DVE) so neither engine falls behind the matmuls
                if len(evs) % 2 == 0:
                    inst = nc.scalar.activation(
                        dst, src,
                        mybir.ActivationFunctionType.Relu if relu
                        else mybir.ActivationFunctionType.Identity,
                        bias=bias,
                    )
                elif relu:
                    inst = nc.vector.tensor_scalar(dst, src, bias, 0.0, add,
                                                   amax)
                else:
                    inst = nc.vector.tensor_scalar_add(dst, src, bias)
                evs.append(inst)
                return inst

            for s in range(nslots):
                cap = caps[s]
                chunks = slot_chunks(s)

                # layer 1, chunk-outer so the first groups only need the
                # first token chunk + one weight m-block
                for n0, nsz in chunks:
                    for m in range(HK):
                        ps = pp.tile([P, 512], f32, tag="ps", bufs=8)
                        for k in range(FK):
                            mm = nc.tensor.matmul(
                                ps[:, :nsz],
                                w1s[s][:, m, k],
                                xts[s][:, k, n0:n0 + nsz],
                                start=(k == 0),
                                stop=(k == FK - 1),
                            )
                            if last_warm is not None:
                                # keep the warm-up burst ahead of the real
                                # stream in the PE queue
                                tile.add_dep_helper(
                                    mm.ins, last_warm.ins,
                                    reason="real MMs follow warm-up burst",
                                )
                                last_warm = None
                        evict(a1s[s][:, m, n0:n0 + nsz], ps[:, :nsz],
                              b1s[:, s, m:m + 1], relu=True)
                        if s == 0 and n0 == 0 and m < 6:
                            for _ in range(2):
                                ps_f = pp.tile([P, 512], f32, tag="ps",
                                               bufs=8)
                                fillers.append(nc.tensor.matmul(
                                    ps_f[:], wm[:, :P], wm[:],
                                    start=True, stop=True))

                # layer 2; split the very last row of the very last slot in
                # half so the final eviction+DMA tail is short
                l2_groups = []
                for n0, nsz in chunks:
                    for m in range(CK):
                        l2_groups.append((m, n0, nsz))
                for m, n0, nsz in l2_groups:
                    ps = pp.tile([P, 512], f32, tag="ps", bufs=8)
                    for k in range(HK):
                        nc.tensor.matmul(
                            ps[:, :nsz],
                            w2s[s][:, m, k],
                            a1s[s][:, k, n0:n0 + nsz],
                            start=(k == 0),
                            stop=(k == HK - 1),
                        )
                    evict(ysb[s][:, m, n0:n0 + nsz], ps[:, :nsz],
                          b2s[:, s, m:m + 1], relu=False)
                    # stream the tile out immediately on the sync ring
                    nc.sync.dma_start(out=yt_r[s][:, m, n0:n0 + nsz],
                                      in_=ysb[s][:, m, n0:n0 + nsz])

            if c0 < caps[0]:
                tile.add_dep_helper(
                    xt0c1_dma.ins, fillers[3].ins,
                    reason="xt0 chunk1 released once the early stream is "
                           "past the first w1 blocks",
                )
            # release the gated bulk DMAs once the ramp is past the
            # corresponding eviction
            for dma, idx in gated:
                tile.add_dep_helper(
                    dma.ins, evs[idx].ins,
                    reason="bulk DMA gated behind ramp-critical phase",
                )

    nc.compile()
    return nc


# ---------------------------------------------------------------------------
# entry point
# ---------------------------------------------------------------------------
def kernel(h, W_mu, b_mu, W_logvar, b_logvar, W1, b1, W2, b2):
    from concourse.bass_utils import run_bass_kernel_spmd

    h = np.ascontiguousarray(np.asarray(h, dtype=np.float32))
    W1 = np.asarray(W1, dtype=np.float32)
    b1 = np.asarray(b1, dtype=np.float32)
    W2 = np.asarray(W2, dtype=np.float32)
    b2 = np.asarray(b2, dtype=np.float32)

    B, F = h.shape
    E, _, H = W1.shape
    C = W2.shape[2]
    assert E % NCORES == 0
    nslots = E // NCORES
    FK, HK, CK = F // P, H // P, C // P

    topk_idx, topk_w = _routing(
        np.asarray(h), np.asarray(W_mu), np.asarray(b_mu),
        np.asarray(W_logvar), np.asarray(b_logvar)
    )

    # per-expert token lists; sort experts by count so each slot's capacity
    # is the max within that slot (slot 0 = busiest experts)
    toks, poss = [], []
    counts = np.zeros(E, np.int64)
    for e in range(E):
        tok, pos = np.nonzero(topk_idx == e)
        toks.append(tok)
        poss.append(pos)
        counts[e] = len(tok)
    perm = np.argsort(-counts, kind="stable")
    caps = []
    for s in range(nslots):
        grp = perm[s * NCORES:(s + 1) * NCORES]
        caps.append(max(64, int(-(-counts[grp].max() // 32) * 32)))

    # gather/dispatch: XgT per expert, padded to its slot's cap; weights
    # swizzled to the [P, mblock, k, 128] device layout
    xt = [np.zeros((NCORES, F, caps[s]), np.float16) for s in range(nslots)]
    w1_in = np.empty((NCORES, nslots, P, HK, FK, P), np.float16)
    w2_in = np.empty((NCORES, nslots, P, CK, HK, P), np.float16)
    b1_in = np.empty((NCORES, P, nslots, HK), np.float32)
    b2_in = np.empty((NCORES, P, nslots, CK), np.float32)
    for i, e in enumerate(perm):
        s, c = divmod(i, NCORES)
        xt[s][c, :, :counts[e]] = h[toks[e]].T.astype(np.float16)
        # [p, mb, k, m] = W[k*P+p, mb*P+m]
        w1_in[c, s] = (W1[e].astype(np.float16)
                       .reshape(FK, P, HK, P).transpose(1, 2, 0, 3))
        w2_in[c, s] = (W2[e].astype(np.float16)
                       .reshape(HK, P, CK, P).transpose(1, 2, 0, 3))
        b1_in[c, :, s, :] = b1[e].reshape(HK, P).T
        b2_in[c, :, s, :] = b2[e].reshape(CK, P).T

    nc = _build_kernel(F, H, C, caps)

    in_maps = []
    for c in range(NCORES):
        m = {"w1": w1_in[c], "w2": w2_in[c], "b1": b1_in[c], "b2": b2_in[c]}
        for s in range(nslots):
            m[f"xt{s}"] = xt[s][c]
        in_maps.append(m)

    trace = bool(os.environ.get("MOE_KERNEL_TRACE"))
    res = run_bass_kernel_spmd(nc, in_maps, list(range(NCORES)), trace=trace)
    global LAST_RESULTS
    LAST_RESULTS = res

    # combine: scatter-add weighted expert outputs
    out = np.zeros((B, C), np.float32)
    for i, e in enumerate(perm):
        s, c = divmod(i, NCORES)
        cnt = counts[e]
        yte = res.results[c][f"yt{s}"]  # [C, cap_s] fp16
        out[toks[e]] += (
            topk_w[toks[e], poss[e]].astype(np.float32)[:, None]
            * yte[:, :cnt].T.astype(np.float32)
        )
    return out


LAST_RESULTS = None


# revision 27
# speedup vs baseline: 1.0774x; 1.0774x over previous
"""Bayesian-router MoE kernel for 8 Trainium2 NeuronCores.

Strategy (expert-parallel, per sharding hint):
  - Router moments / top-k / combine weights: tiny (B*F*E ~ 17 MFLOP), computed
    on host in float64 (min score gap ~1.7e-4, far above fp32 noise, so expert
    selection is stable vs the fp32 reference).
  - Token dispatch: host gathers each expert's routed tokens into a padded,
    transposed buffer XgT [F, CAP] (the host-side equivalent of the
    all-to-all; full I/O contract means shard/unshard happens on host).
    Experts are sorted by token count: the 8 largest go to slot 0 (cap0),
    the 8 smallest to slot 1 (cap1 <= cap0), one of each per core.
  - Device: each of the 8 cores runs its 2-expert MLP on gathered tokens in
    transposed form (A1T = relu(W1^T XgT + b1), YT = W2^T A1T + b2) so no
    on-device transposes are needed. Inputs/activations ship+multiply as fp16
    (PSUM accumulation stays fp32): 1 col/cycle on the PE, 4x fp32 rate.
  - Schedule (rewrite of the ~50us baseline, which had a dense MM stream
    but a ~5us DMA ramp, a ~2.7us HAM cold-clock penalty, and a ~6.4us
    serialized output tail):
      * 8 warm-up matmuls on a memset tile cycle the 8-deep PSUM ring
        (no WAW chaining) and keep the PE busy from the moment the engines
        leave the framework preamble, so the HAM clock-gate is at 8/8
        (2.4 GHz) when the real stream starts instead of ~10us later.
      * weights are host-swizzled to [P, mblock, k, 128] so every weight
        DMA moves >=1KB contiguous runs per partition at line rate, and
        per-m-block transfers complete in the order layer 1 consumes them
        (SDMA round-robins all queued transfers of a ring, so completion
        order tracks issue granularity, and every transfer pays a ~2us
        completion-receipt before its semaphore fires).
      * bulk transfers needed later (second token chunk, W2, all slot-1
        data) are gated behind early evictions so they cannot round-robin-
        steal HBM bandwidth from the ramp-critical pieces.  Completion-
        chaining instead would serialize the ~2us receipts; free-running
        them starves the ramp (both measured).
      * tokens are processed in <=320-column chunks: the L1->L2 boundary
        of a slot then only waits for one chunk's evictions, and eviction/
        output-DMA pipelining stays smooth through the end of the kernel.
      * layer-2 output is evicted as fp16 (halves out-traffic; ~2e-4 extra
        rounding) and each (chunk, m) tile is DMA'd on the otherwise-idle
        sync ring the moment it's evicted -- the baseline queued these
        behind activations on the scalar ring, serializing the tail.
    Remaining fixed costs (measured): ~3.4us engine preamble before the
    first DMA can issue, ~2us DMA completion receipt on the last output
    tile, and ~7us framework postamble that zeroes each engine's ~50-
    semaphore file.  The ~29.3us matmul stream itself is fp16 PE-clock
    bound; the chip's P0 power state (2.0 vs 2.4 GHz, run-to-run) moves
    the total by ~15%.
  - Combine: host scatter-adds w[t,e] * Y_e rows into the output (the
    cross-device reduction of the unshard step).
"""

import os
import numpy as np

NCORES = 8
P = 128
TOP_K = 4
N_WARMUP = 18


# ---------------------------------------------------------------------------
# host-side routing (matches reference math; float64 for stable ordering)
# ---------------------------------------------------------------------------
def _routing(h, W_mu, b_mu, W_logvar, b_logvar):
    h64 = h.astype(np.float64)
    mu = h64 @ W_mu.T.astype(np.float64) + b_mu.astype(np.float64)
    var = (h64 * h64) @ np.exp(W_logvar.astype(np.float64)).T + np.exp(
        b_logvar.astype(np.float64)
    )
    var = np.maximum(var, 1e-12)
    tilde = mu / np.sqrt(1.0 + (np.pi / 8.0) * var)
    t = tilde - tilde.max(axis=1, keepdims=True)
    ex = np.exp(t)
    probs = ex / ex.sum(axis=1, keepdims=True)
    idx = np.argsort(-tilde, axis=1, kind="stable")[:, :TOP_K]
    w = np.take_along_axis(probs, idx, axis=1)
    w = w / np.maximum(w.sum(axis=1, keepdims=True), 1e-12)
    return idx, w


def _chunks(cap, piece=320):
    # chunks of <=piece tokens: fine enough that eviction/DMA pipelining is
    # smooth and the L1->L2 boundary of a slot only waits for one chunk's
    # evictions, coarse enough that matmuls stay streaming-efficient
    n = max(1, -(-cap // piece))
    base, rem = divmod(cap, n)
    out = []
    off = 0
    for i in range(n):
        sz = base + (1 if i < rem else 0)
        out.append((off, sz))
        off += sz
    return out


# ---------------------------------------------------------------------------
# device kernel: 2-expert MLP on pre-gathered transposed tokens
# ---------------------------------------------------------------------------
def _build_kernel(F, H, C, caps):
    import concourse.mybir as mybir
    import concourse.tile as tile
    from concourse import bacc

    f32 = mybir.dt.float32
    f16 = mybir.dt.float16
    FK, HK, CK = F // P, H // P, C // P
    nslots = len(caps)
    # the DMA emission below hardcodes this problem's geometry
    assert (FK, HK, CK, nslots) == (4, 8, 4, 2), (FK, HK, CK, nslots)

    nc = bacc.Bacc("TRN2", target_bir_lowering=False, debug=False,
                   num_devices=NCORES)

    xts_d = [nc.dram_tensor(f"xt{s}", [F, caps[s]], f16, kind="ExternalInput")
             for s in range(nslots)]
    yts_d = [nc.dram_tensor(f"yt{s}", [C, caps[s]], f16, kind="ExternalOutput")
             for s in range(nslots)]
    # weights host-swizzled to [P, mblock, k, 128]: per-partition runs are
    # k*128 contiguous fp16 (>=1KB) and one m-block is an independent 128KB
    # transfer whose arrival unlocks compute for that block.
    w1_d = nc.dram_tensor("w1", [nslots, P, HK, FK, P], f16,
                          kind="ExternalInput")
    w2_d = nc.dram_tensor("w2", [nslots, P, CK, HK, P], f16,
                          kind="ExternalInput")
    b1_d = nc.dram_tensor("b1", [P, nslots, HK], f32, kind="ExternalInput")
    b2_d = nc.dram_tensor("b2", [P, nslots, CK], f32, kind="ExternalInput")

    add, amax = mybir.AluOpType.add, mybir.AluOpType.max

    with tile.TileContext(nc) as tc:
        with (
            tc.tile_pool(name="sb", bufs=1) as sb,
            tc.tile_pool(name="pschunk", bufs=1, space="PSUM") as pp,
        ):
            # --- PE warm-up: junk matmuls from ~7us keep the array busy
            # while the first real operands stream in, so the HAM clock-gate
            # opens to 8/8 right as the real stream begins.  memset FIRST on
            # gpsimd (before the b1/b2 DMA issues, which cost ~0.8us each
            # on that queue).  The warm-ups cycle the SAME 8-deep "ps" PSUM
            # ring the real groups use: no two consecutive warm-ups touch
            # the same bank, so they run back-to-back with no semaphore
            # chaining (a WAW chain would pace them at ~640ns+ each).
            wm = sb.tile([P, 512], f16)
            nc.gpsimd.memset(wm[:], 0.0)
            warms = []
            for _ in range(N_WARMUP):
                ps_w = pp.tile([P, 512], f32, tag="ps", bufs=8)
                warms.append(nc.tensor.matmul(ps_w[:], wm[:, :P], wm[:],
                                              start=True, stop=True))
            last_warm = warms[-1]

            # --- consts (gpsimd SWDGE ring; tiny) ---
            b1s = sb.tile([P, nslots, HK], f32)
            nc.gpsimd.dma_start(out=b1s[:], in_=b1_d[:])
            b2s = sb.tile([P, nslots, CK], f32)
            nc.gpsimd.dma_start(out=b2s[:], in_=b2_d[:])

            # --- SBUF tiles ---
            xts = [sb.tile([P, FK, caps[s]], f16, tag=f"xt{s}",
                           name=f"xts{s}") for s in range(nslots)]
            w1s = [sb.tile([P, HK, FK, P], f16, tag=f"w1_{s}",
                           name=f"w1s{s}") for s in range(nslots)]
            w2s = [sb.tile([P, CK, HK, P], f16, tag=f"w2_{s}",
                           name=f"w2s{s}") for s in range(nslots)]
            a1s = [sb.tile([P, HK, caps[s]], f16, tag=f"a1_{s}",
                           name=f"a1s{s}") for s in range(nslots)]
            ysb = [sb.tile([P, CK, caps[s]], f16, tag=f"yt_{s}",
                           name=f"ysb{s}") for s in range(nslots)]
            xt_r = [xts_d[s].rearrange("(k p) n -> p k n", p=P)
                    for s in range(nslots)]
            yt_r = [yts_d[s].rearrange("(k p) n -> p k n", p=P)
                    for s in range(nslots)]

            # --- input DMAs.  Ramp-critical transfers (first token chunk,
            # first W1 m-blocks) issue immediately in FIFO order, split
            # small so the first matmul group's operands land after ~200KB.
            # Bulk transfers needed later (W2, slot-1 weights/tokens) are
            # GATED behind early layer-1 evictions: completion-chaining
            # them instead would serialize on the ~2us DMA completion
            # receipt per link (measured), and letting them issue freely
            # would steal HBM bandwidth from the ramp.  scalar ring:
            # tokens; sync ring: weights (+ yt evacuations appended
            # later by the compute loop). ---
            gated = []

            def gate(dma, ev_idx):
                gated.append((dma, ev_idx))
                return dma

            c0 = _chunks(caps[0])[0][1]  # first chunk of slot 0
            nc.scalar.dma_start(out=xts[0][:, :2, :c0],
                                in_=xt_r[0][:, :2, :c0])
            nc.sync.dma_start(out=w1s[0][:, 0, 0:2], in_=w1_d[0][:, 0, 0:2])
            nc.scalar.dma_start(out=xts[0][:, 2:, :c0],
                                in_=xt_r[0][:, 2:, :c0])
            nc.sync.dma_start(out=w1s[0][:, 0, 2:4], in_=w1_d[0][:, 0, 2:4])
            # per-m-block w1 pieces: SDMA round-robins among all queued
            # transfers of a ring, so completion order ~ issue order only at
            # matching granularity -- block m lands just before group m
            # needs it
            nc.sync.dma_start(out=w1s[0][:, 1], in_=w1_d[0][:, 1])
            nc.sync.dma_start(out=w1s[0][:, 2], in_=w1_d[0][:, 2])
            nc.sync.dma_start(out=w1s[0][:, 3], in_=w1_d[0][:, 3])
            nc.sync.dma_start(out=w1s[0][:, 4:6], in_=w1_d[0][:, 4:6])
            nc.sync.dma_start(out=w1s[0][:, 6:8], in_=w1_d[0][:, 6:8])
            if c0 < caps[0]:
                # second token chunk of slot 0 is not needed until ~16us:
                # release it mid-warm-up (a Tensor-engine target cannot
                # deadlock the scalar ring this DMA parks on) so it does
                # not round-robin-steal HBM bandwidth from the
                # ramp-critical w1 blocks but still lands in time
                d = nc.scalar.dma_start(out=xts[0][:, :, c0:],
                                        in_=xt_r[0][:, :, c0:])
                tile.add_dep_helper(
                    d.ins, warms[9].ins,
                    reason="xt0 chunk1 released mid-warm-up",
                )
            # gate indices chosen so each bulk release happens after the
            # slot-0 L1 ramp no longer needs exclusive bandwidth, but a few
            # us (transfer + ~2us completion receipt) before the consumer
            # phase starts
            # w2 of slot 0 releases at the very first evictions: the ramp
            # transfers are all done by then, so it gets a clean-bandwidth
            # window and lands (incl. ~2us receipt) well before layer 2.
            gate(nc.sync.dma_start(out=w2s[0][:, 0:2], in_=w2_d[0][:, 0:2]),
                 0)
            gate(nc.sync.dma_start(out=w2s[0][:, 2:4], in_=w2_d[0][:, 2:4]),
                 1)
            for s in range(1, nslots):
                # slot-1 bulk (3.5MB) waits until slot-0 L2 has started
                # (eviction #16 = first L2 eviction) so it never round-robin
                # -steals from w2s0, then streams in fine staggered pieces
                # that complete in consumption order.  All on the sync ring:
                # a gated DMA parked at the head of the scalar ring would
                # block the ACT evictions that share that queue.
                gate(nc.sync.dma_start(out=xts[s][:], in_=xt_r[s][:]), 16)
                gate(nc.sync.dma_start(out=w1s[s][:, 0:2],
                                       in_=w1_d[s][:, 0:2]), 17)
                gate(nc.sync.dma_start(out=w1s[s][:, 2:4],
                                       in_=w1_d[s][:, 2:4]), 18)
                gate(nc.sync.dma_start(out=w1s[s][:, 4:6],
                                       in_=w1_d[s][:, 4:6]), 19)
                gate(nc.sync.dma_start(out=w1s[s][:, 6:8],
                                       in_=w1_d[s][:, 6:8]), 20)
                gate(nc.sync.dma_start(out=w2s[s][:, 0:2],
                                       in_=w2_d[s][:, 0:2]), 22)
                gate(nc.sync.dma_start(out=w2s[s][:, 2:4],
                                       in_=w2_d[s][:, 2:4]), 24)

            # --- compute ---
            evs = []

            def evict(dst, src, bias, relu):
                # alternate PSUM evictions between Scalar(ACT) and
                # Vector(DVE) so neither engine falls behind the matmuls
                if len(evs) % 2 == 0:
                    inst = nc.scalar.activation(
                        dst, src,
                        mybir.ActivationFunctionType.Relu if relu
                        else mybir.ActivationFunctionType.Identity,
                        bias=bias,
                    )
                elif relu:
                    inst = nc.vector.tensor_scalar(dst, src, bias, 0.0, add,
                                                   amax)
                else:
                    inst = nc.vector.tensor_scalar_add(dst, src, bias)
                evs.append(inst)
                return inst

            for s in range(nslots):
                cap = caps[s]
                chunks = _chunks(cap)

                # layer 1, chunk-outer so the first groups only need the
                # first token chunk + one weight m-block
                for n0, nsz in chunks:
                    for m in range(HK):
                        ps = pp.tile([P, 512], f32, tag="ps", bufs=8)
                        for k in range(FK):
                            mm = nc.tensor.matmul(
                                ps[:, :nsz],
                                w1s[s][:, m, k],
                                xts[s][:, k, n0:n0 + nsz],
                                start=(k == 0),
                                stop=(k == FK - 1),
                            )
                            if last_warm is not None:
                                # keep the warm-up burst ahead of the real
                                # stream in the PE queue
                                tile.add_dep_helper(
                                    mm.ins, last_warm.ins,
                                    reason="real MMs follow warm-up burst",
                                )
                                last_warm = None
                        evict(a1s[s][:, m, n0:n0 + nsz], ps[:, :nsz],
                              b1s[:, s, m:m + 1], relu=True)

                # layer 2; split the very last row of the very last slot in
                # half so the final eviction+DMA tail is short
                l2_groups = []
                for n0, nsz in chunks:
                    for m in range(CK):
                        l2_groups.append((m, n0, nsz))
                for m, n0, nsz in l2_groups:
                    ps = pp.tile([P, 512], f32, tag="ps", bufs=8)
                    for k in range(HK):
                        nc.tensor.matmul(
                            ps[:, :nsz],
                            w2s[s][:, m, k],
                            a1s[s][:, k, n0:n0 + nsz],
                            start=(k == 0),
                            stop=(k == HK - 1),
                        )
                    evict(ysb[s][:, m, n0:n0 + nsz], ps[:, :nsz],
                          b2s[:, s, m:m + 1], relu=False)
                    # stream the tile out immediately on the sync ring
                    nc.sync.dma_start(out=yt_r[s][:, m, n0:n0 + nsz],
                                      in_=ysb[s][:, m, n0:n0 + nsz])

            # release the gated bulk DMAs once the ramp is past the
            # corresponding eviction
            for dma, idx in gated:
                tile.add_dep_helper(
                    dma.ins, evs[idx].ins,
                    reason="bulk DMA gated behind ramp-critical phase",
                )

    nc.compile()
    return nc


# ---------------------------------------------------------------------------
# entry point
# ---------------------------------------------------------------------------
def kernel(h, W_mu, b_mu, W_logvar, b_logvar, W1, b1, W2, b2):
    from concourse.bass_utils import run_bass_kernel_spmd

    h = np.ascontiguousarray(np.asarray(h, dtype=np.float32))
    W1 = np.asarray(W1, dtype=np.float32)
    b1 = np.asarray(b1, dtype=np.float32)
    W2 = np.asarray(W2, dtype=np.float32)
    b2 = np.asarray(b2, dtype=np.float32)

    B, F = h.shape
    E, _, H = W1.shape
    C = W2.shape[2]
    assert E % NCORES == 0
    nslots = E // NCORES
    FK, HK, CK = F // P, H // P, C // P

    topk_idx, topk_w = _routing(
        np.asarray(h), np.asarray(W_mu), np.asarray(b_mu),
        np.asarray(W_logvar), np.asarray(b_logvar)
    )

    # per-expert token lists; sort experts by count so each slot's capacity
    # is the max within that slot (slot 0 = busiest experts)
    toks, poss = [], []
    counts = np.zeros(E, np.int64)
    for e in range(E):
        tok, pos = np.nonzero(topk_idx == e)
        toks.append(tok)
        poss.append(pos)
        counts[e] = len(tok)
    perm = np.argsort(-counts, kind="stable")
    caps = []
    for s in range(nslots):
        grp = perm[s * NCORES:(s + 1) * NCORES]
        caps.append(max(64, int(-(-counts[grp].max() // 32) * 32)))

    # gather/dispatch: XgT per expert, padded to its slot's cap; weights
    # swizzled to the [P, mblock, k, 128] device layout
    xt = [np.zeros((NCORES, F, caps[s]), np.float16) for s in range(nslots)]
    w1_in = np.empty((NCORES, nslots, P, HK, FK, P), np.float16)
    w2_in = np.empty((NCORES, nslots, P, CK, HK, P), np.float16)
    b1_in = np.empty((NCORES, P, nslots, HK), np.float32)
    b2_in = np.empty((NCORES, P, nslots, CK), np.float32)
    for i, e in enumerate(perm):
        s, c = divmod(i, NCORES)
        xt[s][c, :, :counts[e]] = h[toks[e]].T.astype(np.float16)
        # [p, mb, k, m] = W[k*P+p, mb*P+m]
        w1_in[c, s] = (W1[e].astype(np.float16)
                       .reshape(FK, P, HK, P).transpose(1, 2, 0, 3))
        w2_in[c, s] = (W2[e].astype(np.float16)
                       .reshape(HK, P, CK, P).transpose(1, 2, 0, 3))
        b1_in[c, :, s, :] = b1[e].reshape(HK, P).T
        b2_in[c, :, s, :] = b2[e].reshape(CK, P).T

    nc = _build_kernel(F, H, C, caps)

    in_maps = []
    for c in range(NCORES):
        m = {"w1": w1_in[c], "w2": w2_in[c], "b1": b1_in[c], "b2": b2_in[c]}
        for s in range(nslots):
            m[f"xt{s}"] = xt[s][c]
        in_maps.append(m)

    trace = bool(os.environ.get("MOE_KERNEL_TRACE"))
    res = run_bass_kernel_spmd(nc, in_maps, list(range(NCORES)), trace=trace)
    global LAST_RESULTS
    LAST_RESULTS = res

    # combine: scatter-add weighted expert outputs
    out = np.zeros((B, C), np.float32)
    for i, e in enumerate(perm):
        s, c = divmod(i, NCORES)
        cnt = counts[e]
        yte = res.results[c][f"yt{s}"]  # [C, cap_s] fp16
        out[toks[e]] += (
            topk_w[toks[e], poss[e]].astype(np.float32)[:, None]
            * yte[:, :cnt].T.astype(np.float32)
        )
    return out


LAST_RESULTS = None
